# revision 1
# baseline (speedup 1.0000x reference)
"""Trainium2 Bass kernel for BasisEncoder: out = one_hot((x % 256) % 64, 64) as f32.

Sharding: pure data parallel over 8 NeuronCores — each core takes a
contiguous 131072-sample shard of x, computes its [131072, 64] f32 slice
of the output, and the host concatenates the shards.

Per-core kernel (memory-bound: 32 MB of output per core):
  - partition p owns the contiguous per-partition sample range
    [p*1024, (p+1)*1024) of the shard, so the x load and all output
    tiles are >=512 B/descriptor DMAs.
  - x loads in 2 chunks (small first chunk so compute starts early);
    idx = x & 63 on the DVE ((x % 256) % 64 == x & 63 for x >= 0).
  - per 16-sample-column tile: one DVE tensor_tensor
        oh[p, f*64 + j] = (idx[p, f] == iota_j)   (int32 cmp -> f32 0/1)
    with a stride-0 broadcast AP on idx against a full-width [128, F*64]
    iota tile (vanilla unit-stride in1; the broadcast costs the same in
    the model but fewer exotic APs in the hot op), then one 512 KB HWDGE
    DMA of the tile to HBM.
  - DVE busy (~73 us) hides under the DMA wall (~95 us at 360 GB/s);
    TimelineSim: 100,467 ns/core = 94.7 us DMA busy + 2.0 us lead-in +
    1.5 us tail + 2.3 us issue-pipe gaps (per-DMA HWDGE 625 + DGE 650
    latency exposed on the first compute-gated output DMA).
"""

import os
import subprocess
import sys
import tempfile
import time

import numpy as np

import concourse.mybir as mybir
from concourse import bacc
from concourse.bass_utils import run_bass_kernel_spmd
from concourse.tile import TileContext

P = 128  # SBUF partitions
NQ = 64  # one-hot width
N_CORES = 8
B_FULL = 1048576
B_SHARD = B_FULL // N_CORES  # 131072 samples per core
K = B_SHARD // P  # 1024 samples per partition
F = 16  # samples (columns) per tile -> [128, 1024] f32 tiles, 512 KB
OH_BUFS = 12
X_SCHED = (32, 992)  # x load chunking (columns)

# Knobs test.py can override (kernel.py itself never reads problem files).
RUN_KWARGS: dict = {}
LAST_RESULTS = None

_cache: dict = {}


def _build() -> bacc.Bacc:
    nc = bacc.Bacc("TRN2", target_bir_lowering=False)
    ntiles = K // F
    x = nc.dram_tensor("x", [B_SHARD], mybir.dt.int32, kind="ExternalInput")
    out = nc.dram_tensor("out", [B_SHARD, NQ], mybir.dt.float32, kind="ExternalOutput")
    x_lay = x[:].rearrange("(p k) -> p k", p=P)
    out_lay = out[:].flatten().rearrange("(p k j) -> p (k j)", p=P, k=K, j=NQ)

    with TileContext(nc) as tc:
        with (
            tc.tile_pool(name="const", bufs=1) as cpool,
            tc.tile_pool(name="oh", bufs=OH_BUFS) as ohpool,
        ):
            iota = cpool.tile([P, F * NQ], mybir.dt.int32)
            nc.gpsimd.iota(
                iota[:], pattern=[[0, F], [1, NQ]], base=0, channel_multiplier=0
            )
            x_all = cpool.tile([P, K], mybir.dt.int32)
            idx_all = cpool.tile([P, K], mybir.dt.int32)
            st = 0
            for kc in X_SCHED:
                sl = slice(st, st + kc)
                nc.sync.dma_start(x_all[:, sl], x_lay[:, sl])
                # DVE only: walrus rejects TensorScalarPtr on Pool/gpsimd
                nc.vector.tensor_scalar(
                    idx_all[:, sl], x_all[:, sl], 63, None, mybir.AluOpType.bitwise_and
                )
                st += kc
            assert st == K
            for t in range(ntiles):
                c0 = t * F
                oh = ohpool.tile([P, F * NQ], mybir.dt.float32, tag="oh")
                nc.vector.tensor_tensor(
                    out=oh[:].rearrange("p (f j) -> p f j", j=NQ),
                    in0=idx_all[:, c0 : c0 + F].unsqueeze(2).broadcast_to([P, F, NQ]),
                    in1=iota[:].rearrange("p (f j) -> p f j", j=NQ),
                    op=mybir.AluOpType.is_equal,
                )
                nc.sync.dma_start(out_lay[:, c0 * NQ : (c0 + F) * NQ], oh[:])
    nc.compile()
    return nc


def kernel(x) -> np.ndarray:
    global LAST_RESULTS
    xv = np.asarray(x)
    assert xv.shape == (B_FULL,), xv.shape
    # Only the low 6 bits matter ((x%256)%64 == x&63 for x >= 0); inputs are
    # < 100000 so an int32 cast is lossless regardless of incoming dtype.
    xv = np.ascontiguousarray(xv.astype(np.int32, copy=False))

    if "nc" not in _cache:
        _cache["nc"] = _build()
    nc = _cache["nc"]

    in_maps = [
        {"x": np.ascontiguousarray(xv[i * B_SHARD : (i + 1) * B_SHARD])}
        for i in range(N_CORES)
    ]
    last_exc = None
    for attempt in range(3):  # transient NRT device errors clear on retry
        try:
            res = run_bass_kernel_spmd(
                nc, in_maps, core_ids=list(range(N_CORES)), **RUN_KWARGS
            )
            break
        except Exception as e:  # noqa: BLE001
            last_exc = e
            # A wedged core (NRT_EXEC_UNIT_UNRECOVERABLE) stays broken for
            # the current PJRT client; drop it so the retry re-opens devices.
            try:
                import jax
                import jax.extend.backend

                jax.clear_caches()
                jax.extend.backend.clear_backends()
            except Exception:  # noqa: BLE001
                pass
            time.sleep(2.0 * (attempt + 1))
    else:
        # A real wedge can outlive in-process backend resets but clears on
        # process restart (fresh PJRT connection). Last resort: run once in
        # a subprocess. Guarded against recursion via env flag.
        if os.environ.get("_BASIS_KERNEL_CHILD") == "1":
            raise last_exc
        return _kernel_subprocess(xv, last_exc)
    LAST_RESULTS = res
    return np.concatenate([r["out"] for r in res.results], axis=0)


def _kernel_subprocess(xv: np.ndarray, parent_exc) -> np.ndarray:
    with tempfile.TemporaryDirectory() as td:
        xp, op = os.path.join(td, "x.npy"), os.path.join(td, "out.npy")
        np.save(xp, xv)
        code = (
            "import sys, numpy as np\n"
            f"sys.path.insert(0, {os.path.dirname(os.path.abspath(__file__))!r})\n"
            "import kernel\n"
            f"out = kernel.kernel(x=np.load({xp!r}))\n"
            f"np.save({op!r}, out)\n"
        )
        try:
            subprocess.run(
                [sys.executable, "-c", code],
                env={**os.environ, "_BASIS_KERNEL_CHILD": "1"},
                check=True,
                timeout=900,
            )
            return np.load(op)
        except Exception as child_exc:
            raise parent_exc from child_exc

